# revision 19
# baseline (speedup 1.0000x reference)
"""Distributed exact-kNN kernel for Trainium2 (8 NeuronCores).

Problem: B=2048 queries (512-d), N=100000 fitted rows, k=5 nearest
neighbors by squared L2; output = mean of the 5 neighbor vectors.

Strategy (shard X_fit along N, 12500 rows/core; norm-sorted classes):
  - Host quantizes to INTEGER-valued fp8: qd = fp8(rint(16 q)),
    xd = fp8(rint(8 x)).  The device matmul accumulates an EXACT
    integer dot in f32 PSUM: dot = qd.xd = 128*(q.x) + quant-noise.
    fp8 DoubleRow covers K=256 per pass -> 2 passes for K=512; the
    moving operand streams ~1 output column/cycle, so the PE floor is
    16 qb * 12500 cols * 2 passes = 400000 cycles ~= 167 us @2.4GHz.
  - Host SORTS each core's shard by ||x||^2 and assigns sorted rank s
    to class g = s//5 (member j = s%5).  Each class of 5 rows has
    near-identical ||x||^2, so the -||x||^2 distance term is a
    per-class constant added on the HOST - no device work at all (the
    old kernel burned 1/3 of PE time on a K=1 matmul adding it
    per-row).  No pad columns: pieces 0-3 hold 512 classes of width
    512, piece 4 holds 452 classes of width 452 (4*2560+2260 = 12500).
  - Device epilogue per (query-block, piece): ACT Copy-drains pB
    (members 2,3) and pC (member 4) to int16; DVE folds pA (members
    0,1) against them directly from PSUM with one wide mixed
    tensor_tensor(max), then two int16 folds -> 512 slot maxima
    shipped back as int16 (engine work sits just under the PE floor).
  - Host: score slots as slotmax_dot + r_class (r_class = 64*(512 -
    min-member ||x||^2), exact ints), global top-24 slots per query
    over all cores*pieces, expand each winning class (5 rows), exact
    f32 re-rank, top-k, mean.  Exact re-rank makes quantization noise
    irrelevant as long as the true top-5 survive in the candidate set
    (validated: 0/2048 rows mismatch; intra-class r spread is a few
    m-units vs ~233 quant-noise sigma).
"""

import sys

if "/opt/trn_rl_repo" not in sys.path:
    sys.path.insert(0, "/opt/trn_rl_repo")

import numpy as np
import ml_dtypes

# ---- problem geometry (hardcoded per spec) ----
B = 2048  # queries
D = 512  # feature dim
N = 100000  # fitted rows
NCORES = 8
NSHARD = N // NCORES  # 12500
QB = 128  # queries per block
NQB = B // QB  # 16
DCH = D // 128  # 4 contraction chunks
MEMB = 5  # members per class (fold arity)
NPIECES = 5
# per-piece member width W: piece cols = 5*W; slots (classes) = W
PIECE_W = [512, 512, 512, 512, 452]  # 5*(4*512+452) = 12500 = NSHARD
PIECE_LO = [0, 2560, 5120, 7680, 10240]  # piece start column
SLOT_LO = [0, 512, 1024, 1536, 2048]  # piece start slot (class) id
NSLOTS = 2500  # total classes per core = NSHARD // MEMB

AQ = 16.0  # query pre-scale before integer rounding
AX = 8.0  # fitted-row pre-scale
RSCALE = 64.0  # host score = dot + rint(64*(512-||x||^2)) of the class

REPEAT = 1  # dev: run the whole pipeline N times (for overhead-cancelling timing)
HWLOOP = False  # dev: use a For_i hardware loop for REPEAT (small NEFF, big trip)
ABLATE = 0  # dev: 0=full, 1=matmuls only
DEDUP_LDW = False  # dev: drop redundant same-weights InstLdweights post-compile
SPOOL_BUFS = 6
UPOOL_BUFS = 6
VPOOL_BUFS = 3

_compiled = None


def _build():
    import concourse.mybir as mybir
    import concourse.tile as tile
    from concourse import bacc

    nc = bacc.Bacc(None, target_bir_lowering=False)

    fp8 = mybir.dt.float8e4
    f32 = mybir.dt.float32
    i16 = mybir.dt.int16
    qT = nc.dram_tensor("qT", [DCH, 128, B], fp8, kind="ExternalInput")
    xT = nc.dram_tensor("xT", [DCH, 128, NSHARD], fp8, kind="ExternalInput")
    cand = nc.dram_tensor("cand", [NQB, QB, NSLOTS], i16, kind="ExternalOutput")

    Copy = mybir.ActivationFunctionType.Copy
    Max = mybir.AluOpType.max
    DR = mybir.MatmulPerfMode.DoubleRow

    with tile.TileContext(nc) as tc:
        with (
            tc.tile_pool(name="persist", bufs=1) as pp,
            tc.tile_pool(name="spool", bufs=SPOOL_BUFS) as sp,
            tc.tile_pool(name="upool", bufs=UPOOL_BUFS) as up,
            tc.tile_pool(name="vpool", bufs=VPOOL_BUFS) as vp,
            tc.tile_pool(name="psA", bufs=2, space="PSUM") as psa,
            tc.tile_pool(name="psB", bufs=1, space="PSUM") as psb,
            tc.tile_pool(name="psC", bufs=2, space="PSUM") as psc,
        ):
            qT_t = pp.tile([128, DCH, B], fp8, name="qTt")
            nc.sync.dma_start(qT_t[:], qT[:].rearrange("c p b -> p c b"))
            xT_t = pp.tile([128, DCH, NSHARD], fp8, name="xTt")
            nc.sync.dma_start(xT_t[:], xT[:].rearrange("c p n -> p c n"))

            def _rep_body():
                for qb in range(NQB):
                    qcols = slice(qb * QB, (qb + 1) * QB)
                    vq = vp.tile([QB, NSLOTS], i16, tag="vq", name="vq")
                    for p in range(NPIECES):
                        W = PIECE_W[p]
                        lo = PIECE_LO[p]
                        pA = psa.tile([QB, 1024], f32, tag="psa", name="psA")
                        pB = psb.tile([QB, 1024], f32, tag="psb", name="psB")
                        pC = psc.tile([QB, 512], f32, tag="psc", name="psC")
                        # member j occupies piece cols [j*W, (j+1)*W):
                        # m0,m1 -> pA; m2,m3 -> pB; m4 -> pC.  PSUM offsets
                        # stay 512-aligned (bank boundary) even when W=452.
                        # pB is single-buffered (8-bank budget): order the
                        # passes so pB finishes early and its ACT drain
                        # frees it before the next iteration needs it.
                        cA0 = (pA, 0, 0)
                        cA1 = (pA, 512, W)
                        cB0 = (pB, 0, 2 * W)
                        cB1 = (pB, 512, 3 * W)
                        cC = (pC, 0, 4 * W)
                        order = {
                            0: [cA0, cA1, cC, cB0, cB1],
                            2: [cB0, cB1, cA0, cA1, cC],
                        }
                        for kk in range(0, DCH, 2):
                            for t, sub, off in order[kk]:
                                nc.tensor.matmul(
                                    t[:, sub : sub + W],
                                    qT_t[:, kk : kk + 2, qcols],
                                    xT_t[:, kk : kk + 2, lo + off : lo + off + W],
                                    start=(kk == 0),
                                    stop=(kk + 2 >= DCH),
                                    perf_mode=DR,
                                )
                        if ABLATE == 1:
                            continue
                        # ACT drains m2,m3 (pB) and m4 (pC); DVE folds pA
                        # (m0,m1) against them directly from PSUM.  ACT ops
                        # have ~300ns fixed cost each, so fewer+bigger wins.
                        sB = sp.tile([QB, 2, 512], i16, tag="sB", name="sB")
                        sC = sp.tile([QB, 512], i16, tag="sC", name="sC")
                        pB2 = pB[:].rearrange("q (m w) -> q m w", m=2)
                        pA2 = pA[:].rearrange("q (m w) -> q m w", m=2)
                        nc.scalar.activation(
                            out=sB[:, :, 0:W], in_=pB2[:, :, 0:W], func=Copy
                        )
                        nc.scalar.activation(
                            out=sC[:, 0:W], in_=pC[:, 0:W], func=Copy
                        )
                        t12 = up.tile([QB, 2, 512], i16, tag="t12", name="t12")
                        nc.vector.tensor_tensor(
                            t12[:, :, 0:W], pA2[:, :, 0:W], sB[:, :, 0:W], Max
                        )  # (m0,m1) vs (m2,m3) in one mixed op; frees pA fast
                        t3 = up.tile([QB, 512], i16, tag="t3", name="t3")
                        nc.vector.tensor_tensor(
                            t3[:, 0:W], t12[:, 0, 0:W], t12[:, 1, 0:W], Max
                        )
                        s0 = SLOT_LO[p]
                        nc.vector.tensor_tensor(
                            vq[:, s0 : s0 + W], t3[:, 0:W], sC[:, 0:W], Max
                        )
                    if ABLATE != 1:
                        nc.sync.dma_start(cand[qb], vq[:])

            if HWLOOP:
                with tc.For_i(0, REPEAT):
                    _rep_body()
            else:
                for rep in range(REPEAT):
                    _rep_body()
    nc.compile()
    if DEDUP_LDW:
        _dedup_ldweights(nc)
    return nc


def _dedup_ldweights(nc):
    import concourse.mybir as mybir

    def key(inst):
        ap = inst.ins[0]
        return (ap.memref, ap.offset, tuple(tuple(x) for x in ap.ap), str(ap.dtype))

    for func in nc.m.functions:
        for block in func.blocks:
            last = None
            keep = []
            for inst in block.instructions:
                if isinstance(inst, mybir.InstLdweights):
                    k = key(inst)
                    si = inst.sync_info
                    clean = si is None or (not si.on_wait and not si.on_update)
                    if last == k and clean:
                        continue
                    last = k
                keep.append(inst)
            block.instructions[:] = keep


def _get_compiled():
    global _compiled
    if _compiled is None:
        _compiled = _build()
    return _compiled


def _col_of_rank():
    """Device column for each sorted rank s (vectorized, length NSHARD)."""
    s = np.arange(NSHARD)
    g, j = s // MEMB, s % MEMB
    col = np.empty(NSHARD, dtype=np.int64)
    for p in range(NPIECES):
        W = PIECE_W[p]
        m = (g >= SLOT_LO[p]) & (g < SLOT_LO[p] + W)
        col[m] = PIECE_LO[p] + (g[m] - SLOT_LO[p]) + W * j[m]
    return col


_COLS = _col_of_rank()


def _sort_shard(Xi):
    """Per-core norm-sorted layout.

    Returns (xT_np [DCH,128,NSHARD] fp8, order [NSHARD] rank->row,
    r_cls [NSLOTS] int64 per-class score offset)."""
    fp8 = ml_dtypes.float8_e4m3fn
    xsq = np.einsum("nd,nd->n", Xi, Xi, dtype=np.float32)
    order = np.argsort(xsq, kind="stable")
    xd = np.rint(AX * Xi[order]).astype(fp8)  # [NSHARD, D] sorted by norm
    X_cols = np.empty((NSHARD, D), dtype=fp8)
    X_cols[_COLS] = xd
    xT_np = np.ascontiguousarray(X_cols.T.reshape(DCH, 128, NSHARD))
    r = np.rint(RSCALE * (512.0 - xsq[order].astype(np.float64))).astype(np.int64)
    r_cls = r[0 : NSLOTS * MEMB : MEMB]  # ascending xsq -> member 0 has max r
    return xT_np, order, r_cls


def _prepare_inputs(q, X):
    """Build per-core in_maps. q: [B, D] f32, X: [N, D] f32."""
    fp8 = ml_dtypes.float8_e4m3fn
    qd = np.rint(AQ * q).astype(fp8)  # [B, D] integer-valued fp8
    qT_np = np.ascontiguousarray(qd.T.reshape(DCH, 128, B))
    in_maps = []
    for core in range(NCORES):
        Xi = X[core * NSHARD : (core + 1) * NSHARD]
        xT_np, _, _ = _sort_shard(Xi)
        in_maps.append({"qT": qT_np, "xT": xT_np})
    return in_maps


def _run_device(in_maps, trace=False, tmpdir=None):
    from concourse.bass_utils import run_bass_kernel_spmd

    nc = _get_compiled()
    kwargs = {}
    if trace:
        kwargs = {"trace": True, "tmpdir": tmpdir}
    return run_bass_kernel_spmd(nc, in_maps, core_ids=list(range(NCORES)), **kwargs)


def _merge_host(cand_all, q, X, k, orders, r_cls_all):
    """cand_all: [NCORES, NQB, QB, NSLOTS] i16 slot-max dots.

    Returns [B, 1, D] f32."""
    S = cand_all.astype(np.int64).reshape(NCORES, B, NSLOTS)
    S = S + r_cls_all[:, None, :]  # add per-class -||x||^2 term
    S = np.moveaxis(S, 0, 1).reshape(B, NCORES * NSLOTS)

    C = max(24, 4 * k)
    top = np.argpartition(-S, C, axis=1)[:, :C]  # [B, C] global slot ids
    core_t, g_t = np.divmod(top, NSLOTS)
    ranks = g_t[:, :, None] * MEMB + np.arange(MEMB)[None, None, :]  # [B, C, 5]
    ord_all = np.stack(orders)  # [NCORES, NSHARD] rank -> row id in shard
    rows = ord_all[core_t[:, :, None], ranks] + core_t[:, :, None] * NSHARD
    rows = rows.reshape(B, C * MEMB)

    out = np.empty((B, D), dtype=np.float32)
    CH = 512
    for i in range(0, B, CH):
        rr = rows[i : i + CH]
        Xg = X[rr]  # [CH, C*5, D]
        xsq_g = np.einsum("bcd,bcd->bc", Xg, Xg, dtype=np.float32)
        d2 = xsq_g - 2.0 * np.einsum("bcd,bd->bc", Xg, q[i : i + CH], dtype=np.float32)
        win = np.argpartition(d2, k - 1, axis=1)[:, :k]
        neigh = np.take_along_axis(Xg, win[:, :, None], axis=1)
        out[i : i + CH] = neigh.mean(axis=1, dtype=np.float32)
    return out.reshape(B, 1, D).astype(np.float32)


def kernel(x_enc, X_fit, n_neighbors, _trace=False, _tmpdir=None):
    q = np.asarray(x_enc, dtype=np.float32).reshape(B, D)
    X = np.asarray(X_fit, dtype=np.float32)
    k = int(n_neighbors)
    fp8 = ml_dtypes.float8_e4m3fn
    qd = np.rint(AQ * q).astype(fp8)
    qT_np = np.ascontiguousarray(qd.T.reshape(DCH, 128, B))
    in_maps, orders, r_cls_all = [], [], []
    for core in range(NCORES):
        Xi = X[core * NSHARD : (core + 1) * NSHARD]
        xT_np, order, r_cls = _sort_shard(Xi)
        in_maps.append({"qT": qT_np, "xT": xT_np})
        orders.append(order)
        r_cls_all.append(r_cls)
    res = _run_device(in_maps, trace=_trace, tmpdir=_tmpdir)
    cand_all = np.stack([res.results[c]["cand"] for c in range(NCORES)])
    out = _merge_host(cand_all, q, X, k, orders, np.stack(r_cls_all))
    if _trace:
        return out, res
    return out


# revision 21
# speedup vs baseline: 1.0107x; 1.0107x over previous
"""Distributed exact-kNN kernel for Trainium2 (8 NeuronCores).

Problem: B=2048 queries (512-d), N=100000 fitted rows, k=5 nearest
neighbors by squared L2; output = mean of the 5 neighbor vectors.

Strategy (shard X_fit along N, 12500 rows/core; norm-sorted classes):
  - Host quantizes to INTEGER-valued fp8: qd = fp8(rint(16 q)),
    xd = fp8(rint(8 x)).  The device matmul accumulates an EXACT
    integer dot in f32 PSUM: dot = qd.xd = 128*(q.x) + quant-noise.
    fp8 DoubleRow covers K=256 per pass -> 2 passes for K=512; the
    moving operand streams ~1 output column/cycle, so the PE floor is
    16 qb * 12500 cols * 2 passes = 400000 cycles ~= 167 us @2.4GHz.
  - Host SORTS each core's shard by ||x||^2 and assigns sorted rank s
    to class g = s//5 (member j = s%5).  Each class of 5 rows has
    near-identical ||x||^2, so the -||x||^2 distance term is a
    per-class constant added on the HOST - no device work at all (the
    old kernel burned 1/3 of PE time on a K=1 matmul adding it
    per-row).  No pad columns: pieces 0-3 hold 512 classes of width
    512, piece 4 holds 452 classes of width 452 (4*2560+2260 = 12500).
  - Device epilogue per (query-block, piece): ACT Copy-drains pB
    (members 2,3) and pC (member 4) to int16; DVE folds pA (members
    0,1) against them directly from PSUM with one wide mixed
    tensor_tensor(max), then two int16 folds -> 512 slot maxima
    shipped back as int16 (engine work sits just under the PE floor).
  - Host: score slots as slotmax_dot + r_class (r_class = 64*(512 -
    min-member ||x||^2), exact ints), global top-24 slots per query
    over all cores*pieces, expand each winning class (5 rows), exact
    f32 re-rank, top-k, mean.  Exact re-rank makes quantization noise
    irrelevant as long as the true top-5 survive in the candidate set
    (validated: 0/2048 rows mismatch; intra-class r spread is a few
    m-units vs ~233 quant-noise sigma).
"""

import sys

if "/opt/trn_rl_repo" not in sys.path:
    sys.path.insert(0, "/opt/trn_rl_repo")

import numpy as np
import ml_dtypes

# ---- problem geometry (hardcoded per spec) ----
B = 2048  # queries
D = 512  # feature dim
N = 100000  # fitted rows
NCORES = 8
NSHARD = N // NCORES  # 12500
QB = 128  # queries per block
NQB = B // QB  # 16
DCH = D // 128  # 4 contraction chunks
MEMB = 5  # members per class (fold arity)
NPIECES = 5
# per-piece member width W: piece cols = 5*W; slots (classes) = W
PIECE_W = [512, 512, 512, 512, 452]  # 5*(4*512+452) = 12500 = NSHARD
PIECE_LO = [0, 2560, 5120, 7680, 10240]  # piece start column
SLOT_LO = [0, 512, 1024, 1536, 2048]  # piece start slot (class) id
NSLOTS = 2500  # total classes per core = NSHARD // MEMB

AQ = 16.0  # query pre-scale before integer rounding
AX = 8.0  # fitted-row pre-scale
RSCALE = 64.0  # host score = dot + rint(64*(512-||x||^2)) of the class

REPEAT = 1  # dev: run the whole pipeline N times (for overhead-cancelling timing)
HWLOOP = False  # dev: use a For_i hardware loop for REPEAT (small NEFF, big trip)
ABLATE = 0  # dev: 0=full, 1=matmuls only
DEDUP_LDW = 0  # dev: 1=drop redundant same-weights InstLdweights, 2=drop ALL clean ones
SPOOL_BUFS = 6
UPOOL_BUFS = 6
VPOOL_BUFS = 3

_compiled = None


def _build():
    import concourse.mybir as mybir
    import concourse.tile as tile
    from concourse import bacc

    nc = bacc.Bacc(None, target_bir_lowering=False)

    fp8 = mybir.dt.float8e4
    f32 = mybir.dt.float32
    i16 = mybir.dt.int16
    qT = nc.dram_tensor("qT", [DCH, 128, B], fp8, kind="ExternalInput")
    xT = nc.dram_tensor("xT", [DCH, 128, NSHARD], fp8, kind="ExternalInput")
    cand = nc.dram_tensor("cand", [NQB, QB, NSLOTS], i16, kind="ExternalOutput")

    Copy = mybir.ActivationFunctionType.Copy
    Max = mybir.AluOpType.max
    DR = mybir.MatmulPerfMode.DoubleRow

    with tile.TileContext(nc) as tc:
        with (
            tc.tile_pool(name="persist", bufs=1) as pp,
            tc.tile_pool(name="spool", bufs=SPOOL_BUFS) as sp,
            tc.tile_pool(name="upool", bufs=UPOOL_BUFS) as up,
            tc.tile_pool(name="vpool", bufs=VPOOL_BUFS) as vp,
            tc.tile_pool(name="psA", bufs=2, space="PSUM") as psa,
            tc.tile_pool(name="psB", bufs=1, space="PSUM") as psb,
            tc.tile_pool(name="psC", bufs=2, space="PSUM") as psc,
        ):
            qT_t = pp.tile([128, DCH, B], fp8, name="qTt")
            nc.sync.dma_start(qT_t[:], qT[:].rearrange("c p b -> p c b"))
            xT_t = pp.tile([128, DCH, NSHARD], fp8, name="xTt")
            nc.sync.dma_start(xT_t[:], xT[:].rearrange("c p n -> p c n"))

            def _rep_body():
                for qb in range(NQB):
                    qcols = slice(qb * QB, (qb + 1) * QB)
                    vq = vp.tile([QB, NSLOTS], i16, tag="vq", name="vq")
                    for p in range(NPIECES):
                        W = PIECE_W[p]
                        lo = PIECE_LO[p]
                        pA = psa.tile([QB, 1024], f32, tag="psa", name="psA")
                        pB = psb.tile([QB, 1024], f32, tag="psb", name="psB")
                        pC = psc.tile([QB, 512], f32, tag="psc", name="psC")
                        # member j occupies piece cols [j*W, (j+1)*W):
                        # m0,m1 -> pA; m2,m3 -> pB; m4 -> pC.  PSUM offsets
                        # stay 512-aligned (bank boundary) even when W=452.
                        # pB is single-buffered (8-bank budget): order the
                        # passes so pB finishes early and its ACT drain
                        # frees it before the next iteration needs it.
                        cA0 = (pA, 0, 0)
                        cA1 = (pA, 512, W)
                        cB0 = (pB, 0, 2 * W)
                        cB1 = (pB, 512, 3 * W)
                        cC = (pC, 0, 4 * W)
                        order = {
                            0: [cA0, cA1, cC, cB0, cB1],
                            2: [cB0, cB1, cA0, cA1, cC],
                        }
                        for kk in range(0, DCH, 2):
                            for t, sub, off in order[kk]:
                                nc.tensor.matmul(
                                    t[:, sub : sub + W],
                                    qT_t[:, kk : kk + 2, qcols],
                                    xT_t[:, kk : kk + 2, lo + off : lo + off + W],
                                    start=(kk == 0),
                                    stop=(kk + 2 >= DCH),
                                    perf_mode=DR,
                                )
                        if ABLATE == 1:
                            continue
                        # ACT drains m2,m3 (pB) and m4 (pC); DVE folds pA
                        # (m0,m1) against them directly from PSUM.  ACT ops
                        # have ~300ns fixed cost each, so fewer+bigger wins.
                        sB = sp.tile([QB, 2, 512], i16, tag="sB", name="sB")
                        sC = sp.tile([QB, 512], i16, tag="sC", name="sC")
                        pB2 = pB[:].rearrange("q (m w) -> q m w", m=2)
                        pA2 = pA[:].rearrange("q (m w) -> q m w", m=2)
                        nc.scalar.activation(
                            out=sB[:, :, 0:W], in_=pB2[:, :, 0:W], func=Copy
                        )
                        nc.scalar.activation(
                            out=sC[:, 0:W], in_=pC[:, 0:W], func=Copy
                        )
                        t12 = up.tile([QB, 2, 512], i16, tag="t12", name="t12")
                        nc.vector.tensor_tensor(
                            t12[:, :, 0:W], pA2[:, :, 0:W], sB[:, :, 0:W], Max
                        )  # (m0,m1) vs (m2,m3) in one mixed op; frees pA fast
                        t3 = up.tile([QB, 512], i16, tag="t3", name="t3")
                        nc.vector.tensor_tensor(
                            t3[:, 0:W], t12[:, 0, 0:W], t12[:, 1, 0:W], Max
                        )
                        s0 = SLOT_LO[p]
                        nc.vector.tensor_tensor(
                            vq[:, s0 : s0 + W], t3[:, 0:W], sC[:, 0:W], Max
                        )
                    if ABLATE != 1:
                        nc.sync.dma_start(cand[qb], vq[:])

            if HWLOOP:
                with tc.For_i(0, REPEAT):
                    _rep_body()
            else:
                for rep in range(REPEAT):
                    _rep_body()
    nc.compile()
    if DEDUP_LDW:
        _dedup_ldweights(nc)
    return nc


def _dedup_ldweights(nc):
    import concourse.mybir as mybir

    def key(inst):
        ap = inst.ins[0]
        return (ap.memref, ap.offset, tuple(tuple(x) for x in ap.ap), str(ap.dtype))

    for func in nc.m.functions:
        for block in func.blocks:
            last = None
            keep = []
            for inst in block.instructions:
                if isinstance(inst, mybir.InstLdweights):
                    k = key(inst)
                    si = inst.sync_info
                    clean = si is None or (not si.on_wait and not si.on_update)
                    if clean and (DEDUP_LDW == 2 or last == k):
                        continue
                    last = k
                keep.append(inst)
            block.instructions[:] = keep


def _get_compiled():
    global _compiled
    if _compiled is None:
        _compiled = _build()
    return _compiled


def _col_of_rank():
    """Device column for each sorted rank s (vectorized, length NSHARD)."""
    s = np.arange(NSHARD)
    g, j = s // MEMB, s % MEMB
    col = np.empty(NSHARD, dtype=np.int64)
    for p in range(NPIECES):
        W = PIECE_W[p]
        m = (g >= SLOT_LO[p]) & (g < SLOT_LO[p] + W)
        col[m] = PIECE_LO[p] + (g[m] - SLOT_LO[p]) + W * j[m]
    return col


_COLS = _col_of_rank()


def _sort_shard(Xi):
    """Per-core norm-sorted layout.

    Returns (xT_np [DCH,128,NSHARD] fp8, order [NSHARD] rank->row,
    r_cls [NSLOTS] int64 per-class score offset)."""
    fp8 = ml_dtypes.float8_e4m3fn
    xsq = np.einsum("nd,nd->n", Xi, Xi, dtype=np.float32)
    order = np.argsort(xsq, kind="stable")
    xd = np.rint(AX * Xi[order]).astype(fp8)  # [NSHARD, D] sorted by norm
    X_cols = np.empty((NSHARD, D), dtype=fp8)
    X_cols[_COLS] = xd
    xT_np = np.ascontiguousarray(X_cols.T.reshape(DCH, 128, NSHARD))
    r = np.rint(RSCALE * (512.0 - xsq[order].astype(np.float64))).astype(np.int64)
    r_cls = r[0 : NSLOTS * MEMB : MEMB]  # ascending xsq -> member 0 has max r
    return xT_np, order, r_cls


def _prepare_inputs(q, X):
    """Build per-core in_maps. q: [B, D] f32, X: [N, D] f32."""
    fp8 = ml_dtypes.float8_e4m3fn
    qd = np.rint(AQ * q).astype(fp8)  # [B, D] integer-valued fp8
    qT_np = np.ascontiguousarray(qd.T.reshape(DCH, 128, B))
    in_maps = []
    for core in range(NCORES):
        Xi = X[core * NSHARD : (core + 1) * NSHARD]
        xT_np, _, _ = _sort_shard(Xi)
        in_maps.append({"qT": qT_np, "xT": xT_np})
    return in_maps


def _run_device(in_maps, trace=False, tmpdir=None):
    from concourse.bass_utils import run_bass_kernel_spmd

    nc = _get_compiled()
    kwargs = {}
    if trace:
        kwargs = {"trace": True, "tmpdir": tmpdir}
    return run_bass_kernel_spmd(nc, in_maps, core_ids=list(range(NCORES)), **kwargs)


def _merge_host(cand_all, q, X, k, orders, r_cls_all):
    """cand_all: [NCORES, NQB, QB, NSLOTS] i16 slot-max dots.

    Returns [B, 1, D] f32."""
    S = cand_all.astype(np.int64).reshape(NCORES, B, NSLOTS)
    S = S + r_cls_all[:, None, :]  # add per-class -||x||^2 term
    S = np.moveaxis(S, 0, 1).reshape(B, NCORES * NSLOTS)

    C = max(24, 4 * k)
    top = np.argpartition(-S, C, axis=1)[:, :C]  # [B, C] global slot ids
    core_t, g_t = np.divmod(top, NSLOTS)
    ranks = g_t[:, :, None] * MEMB + np.arange(MEMB)[None, None, :]  # [B, C, 5]
    ord_all = np.stack(orders)  # [NCORES, NSHARD] rank -> row id in shard
    rows = ord_all[core_t[:, :, None], ranks] + core_t[:, :, None] * NSHARD
    rows = rows.reshape(B, C * MEMB)

    out = np.empty((B, D), dtype=np.float32)
    CH = 512
    for i in range(0, B, CH):
        rr = rows[i : i + CH]
        Xg = X[rr]  # [CH, C*5, D]
        xsq_g = np.einsum("bcd,bcd->bc", Xg, Xg, dtype=np.float32)
        d2 = xsq_g - 2.0 * np.einsum("bcd,bd->bc", Xg, q[i : i + CH], dtype=np.float32)
        win = np.argpartition(d2, k - 1, axis=1)[:, :k]
        neigh = np.take_along_axis(Xg, win[:, :, None], axis=1)
        out[i : i + CH] = neigh.mean(axis=1, dtype=np.float32)
    return out.reshape(B, 1, D).astype(np.float32)


def kernel(x_enc, X_fit, n_neighbors, _trace=False, _tmpdir=None):
    q = np.asarray(x_enc, dtype=np.float32).reshape(B, D)
    X = np.asarray(X_fit, dtype=np.float32)
    k = int(n_neighbors)
    fp8 = ml_dtypes.float8_e4m3fn
    qd = np.rint(AQ * q).astype(fp8)
    qT_np = np.ascontiguousarray(qd.T.reshape(DCH, 128, B))
    in_maps, orders, r_cls_all = [], [], []
    for core in range(NCORES):
        Xi = X[core * NSHARD : (core + 1) * NSHARD]
        xT_np, order, r_cls = _sort_shard(Xi)
        in_maps.append({"qT": qT_np, "xT": xT_np})
        orders.append(order)
        r_cls_all.append(r_cls)
    res = _run_device(in_maps, trace=_trace, tmpdir=_tmpdir)
    cand_all = np.stack([res.results[c]["cand"] for c in range(NCORES)])
    out = _merge_host(cand_all, q, X, k, orders, np.stack(r_cls_all))
    if _trace:
        return out, res
    return out
